# revision 19
# baseline (speedup 1.0000x reference)
"""MultiQueryAttention Trainium2 kernel (8 NeuronCores, SPMD).

Reference computation (per batch b):
    q_proj = q @ Wq            [T, C] -> [T, H, D]   (H=16 heads, D=64)
    k_proj = k @ Wk            [T, D]   (single shared KV head)
    v_proj = v @ Wv            [T, D]
    S_h    = q_h @ k_proj.T / sqrt(D)      [T, T] per head
    P      = softmax(S)        (no mask)
    out    = (P @ v_proj  for each head) -> [T, C]; out @ Wp + bp

Sharding: 8 cores = batch (4) x head-halves (2). Each core handles one
batch and 8 query heads; the shared K/V projections are replicated.
Wq is split column-wise, Wp row-wise; each pair of cores produces a
partial [T, C] output that the host sums (+ bp).

Device layout notes:
  - All matmul operands are bf16 (PE streams bf16 at 1 cyc/row vs 2 for
    fp32); PSUM accumulation is fp32.
  - Host pre-transposes q/k/v to [C, T] so every projection contraction
    (over C) has C on the partition axis.
  - Scores are computed transposed: S^T[tk, tq] so that P^T can feed the
    P@V matmul directly as the stationary operand.  The two heads of a
    head-pair run concurrently in the PE array via row tiling (K=64 each,
    base partitions 0 and 64).
  - Row-sums of P come for free from a ones-column appended to v_proj
    (stationary [v | 1] -> output row 64 is the softmax denominator).
  - softmax(x) is computed without max-subtraction: scores are ~N(0, 0.4)
    here so exp is safe in fp32, and the reference's max-subtraction is
    mathematically a no-op.

Schedule notes (v2):
  - The ACT engine is the bottleneck: 256 exp instructions of [128,1024]
    at ~(N+352)/1.2ns each = ~280us busy.  Per chunk we emit
    scores(c) -> exp(c) -> [extras] -> PV(c-1): the PV of the PREVIOUS
    chunk runs after the next scores pair, so the in-order PE queue never
    makes ACT wait for a score pair stuck behind PV matmuls.
  - Input DMAs are ordered by first use at 512-tk granularity, and
    kproj/vproj are emitted per 512-tk block interleaved into block 0, so
    the first exp fires as soon as ~2.5MB (not 6.5MB) has landed.
  - vproj is computed transposed (wv stationary, vt moving, N=512) which
    is 4x fewer PE slots than the naive [tk,d] form; a small SBUF->SBUF
    DMA transpose (plus a column-interleaving DVE copy) restores the
    [tk, d] layout the PV stationary needs.
  - The softmax reciprocal is partition-broadcast on the (otherwise idle)
    GPSIMD engine instead of a DRAM round-trip DMA bounce.
"""

import numpy as np
import ml_dtypes
from contextlib import ExitStack

import concourse.bacc as bacc
import concourse.bass as bass
import concourse.mybir as mybir
import concourse.tile as tile

B, T, C = 4, 2048, 1024
H, D = 16, 64
HPC = 8              # heads per core
HD = HPC * D         # 512 per-core attention output dims
NCORES = 8
P128 = 128
NCC = C // P128      # 8 contraction chunks over C
NTK = T // P128      # 16 key chunks
NTQB = 4             # query blocks of 512
TQB = 512
NTP = 4              # head-pairs per core
SCALE = 1.0 / 8.0    # 1/sqrt(64)

BF = mybir.dt.bfloat16
F32 = mybir.dt.float32
NPBF = ml_dtypes.bfloat16
DEBUG = False      # adds intermediate dumps (k2/v65/qpt/attn) as outputs


def emit_kernel(ctx: ExitStack, tc: tile.TileContext, dr):
    nc = tc.nc
    EXP = mybir.ActivationFunctionType.Exp

    const = ctx.enter_context(tc.tile_pool(name="const", bufs=1))
    persist = ctx.enter_context(tc.tile_pool(name="persist", bufs=1))
    stream = ctx.enter_context(tc.tile_pool(name="stream", bufs=2))
    ppool = ctx.enter_context(tc.tile_pool(name="ppool", bufs=7))
    small = ctx.enter_context(tc.tile_pool(name="small", bufs=2))
    outp = ctx.enter_context(tc.tile_pool(name="outp", bufs=2))
    # PSUM budget (8 banks): s2 rotation 2x2 + pv 2 + qp 1 + po 1
    ps_s2 = ctx.enter_context(tc.tile_pool(name="ps_s2", bufs=2, space="PSUM"))
    ps_pv = ctx.enter_context(tc.tile_pool(name="ps_pv", bufs=1, space="PSUM"))
    ps_qp = ctx.enter_context(tc.tile_pool(name="ps_qp", bufs=1, space="PSUM"))
    ps_po = ctx.enter_context(tc.tile_pool(name="ps_po", bufs=1, space="PSUM"))

    kT_r = dr["kT"].ap().rearrange("(cc p) t -> p cc t", p=P128)
    qT_r = dr["qT"].ap().rearrange("(cc p) t -> p cc t", p=P128)
    vT_r = dr["vT"].ap().rearrange("(cc p) t -> p cc t", p=P128)
    wq_r = dr["wq"].ap().rearrange("(cc p) d -> p cc d", p=P128)

    # ---- input DMAs, ordered by first use ----
    # critical set for the first exp: wk2, wq[:,0:128], kT[:,0:512], qT[:,0:512]
    wk2_sb = const.tile([P128, NCC, P128], BF)       # Wk duplicated -> [*, 128]
    nc.sync.dma_start(wk2_sb, dr["wk2"].ap().rearrange("(cc p) d -> p cc d", p=P128))
    wq_sb = const.tile([P128, NCC, HD], BF)          # [c-in-chunk, cc, dcol]
    nc.sync.dma_start(wq_sb[:, :, 0:P128], wq_r[:, :, 0:P128])
    kts = []
    for cc in range(NCC):
        kt = stream.tile([P128, T], BF, tag="kv_chunk", name=f"kt{cc}", bufs=8)
        nc.sync.dma_start(kt[:, 0:512], kT_r[:, cc, 0:512])
        kts.append(kt)
    qt_sb = persist.tile([P128, NCC, T], BF)
    for cc in range(NCC):
        nc.sync.dma_start(qt_sb[:, cc, 0:TQB], qT_r[:, cc, 0:TQB])
    # block-0 support: vproj data + second kT quarter + wq col 1
    wv_sb = const.tile([P128, NCC, D], BF)
    nc.sync.dma_start(wv_sb, dr["wv"].ap().rearrange("(cc p) d -> p cc d", p=P128))
    vt_sb = stream.tile([P128, NCC, T], BF, tag="vt_all", bufs=1)
    nc.sync.dma_start(vt_sb[:, :, 0:512], vT_r[:, :, 0:512])
    for cc in range(NCC):
        nc.sync.dma_start(kts[cc][:, 512:1024], kT_r[:, cc, 512:1024])
    nc.sync.dma_start(wq_sb[:, :, P128:2 * P128], wq_r[:, :, P128:2 * P128])
    nc.sync.dma_start(vt_sb[:, :, 512:1024], vT_r[:, :, 512:1024])
    for cc in range(NCC):
        nc.sync.dma_start(kts[cc][:, 1024:2048], kT_r[:, cc, 1024:2048])
    nc.sync.dma_start(vt_sb[:, :, 1024:2048], vT_r[:, :, 1024:2048])
    nc.sync.dma_start(wq_sb[:, :, 2 * P128:HD], wq_r[:, :, 2 * P128:HD])
    for cc in range(NCC):
        nc.sync.dma_start(qt_sb[:, cc, TQB:T], qT_r[:, cc, TQB:T])
    wp_sb = const.tile([P128, HD // P128, C], BF)    # [hd-in-chunk, r, c-out]
    nc.sync.dma_start(wp_sb, dr["wp"].ap().rearrange("(r p) c -> p r c", p=P128))

    # ---- K projection: k2[0:64]=k_projT, k2[64:128]=k_projT (dup), one
    # 512-tk block at a time so scores gate at blk granularity ----
    k2_sb = persist.tile([P128, T], BF)

    def kproj_blk_steps(b, pool, tag):
        kps = pool.tile([P128, 512], F32, tag=tag, name=f"kps{b}")

        def mm(cc):
            nc.tensor.matmul(kps, wk2_sb[:, cc, :],
                             kts[cc][:, b * 512:(b + 1) * 512],
                             start=(cc == 0), stop=(cc == NCC - 1))

        def fin():
            nc.vector.tensor_copy(k2_sb[:, b * 512:(b + 1) * 512], kps)
        return [lambda cc=cc: mm(cc) for cc in range(NCC)] + [fin]

    # ---- V projection, transposed: vps[d 64, tk 512] = wv.T @ vt_blk,
    # then a column-interleaving copy + DMA transpose into v65 [tk, d] ----
    v65_sb = persist.tile([P128, NTK, D + 1], BF)
    nc.vector.memset(v65_sb[:, :, D:D + 1], 1.0)
    v2t_sb = persist.tile([D, T], BF)

    def vproj_blk_steps(b):
        vps = ps_po.tile([P128, 512], F32, tag="ps_po", name=f"vps{b}")

        def mm(cc):
            nc.tensor.matmul(vps[0:D, :], wv_sb[:, cc, :],
                             vt_sb[:, cc, b * 512:(b + 1) * 512],
                             start=(cc == 0), stop=(cc == NCC - 1))

        def fin():
            # xbar transpose: out[p, c, d] = v2t[d, c*128+p], but it needs a
            # contiguous dest tile; a small DVE copy lands it in v65's
            # 65-element-pitch layout.
            nc.vector.tensor_copy(v2t_sb[:, b * 512:(b + 1) * 512],
                                  vps[0:D, :])
            vtr = small.tile([P128, 4, D], BF, tag="vtr", name=f"vtr{b}")
            # ACT queue: keeps the transpose off the (in-order) sync queue,
            # which is deep in backpressured bulk input DMAs during block 0
            nc.scalar.dma_start_transpose(
                vtr, v2t_sb[:, b * 512:(b + 1) * 512])
            nc.vector.tensor_copy(v65_sb[:, b * 4:(b + 1) * 4, 0:D], vtr)
        return [lambda cc=cc: mm(cc) for cc in range(NCC)] + [fin]

    # ---- Q projection: one (dcol, tq-block) chain ----
    qpt_sb = persist.tile([P128, NTP, T], BF)

    def qproj_chain(j, tqb):
        qps = ps_qp.tile([P128, 512], F32, tag="ps_qp", name=f"qps_{j}_{tqb}")

        def mm(cc):
            nc.tensor.matmul(
                qps, wq_sb[:, cc, j * P128:(j + 1) * P128],
                qt_sb[:, cc, tqb * 512:(tqb + 1) * 512],
                start=(cc == 0), stop=(cc == NCC - 1))

        def fin():
            nc.vector.tensor_copy(
                qpt_sb[:, j, tqb * 512:(tqb + 1) * 512], qps)
        return [lambda cc=cc: mm(cc) for cc in range(NCC)] + [fin]

    attn_sb = persist.tile([P128, NTP, T], BF)   # attn_outT (normalized), bf16

    def wp_tile(tt):
        # two sequential half-chains through one PSUM bank
        po = ps_po.tile([P128, 512], F32, tag="ps_po", name=f"po_{tt}")
        os_ = outp.tile([P128, 1024], F32, tag="os", name=f"os_{tt}")
        steps = []
        for half in range(2):
            for rr in range(HD // P128):
                def mm(rr=rr, half=half):
                    nc.tensor.matmul(
                        po, attn_sb[:, rr, tt * P128:(tt + 1) * P128],
                        wp_sb[:, rr, half * 512:half * 512 + 512],
                        start=(rr == 0), stop=(rr == 3))
                steps.append(mm)

            def cp(half=half):
                nc.vector.tensor_copy(os_[:, half * 512:half * 512 + 512], po)
            steps.append(cp)

        def out(tt=tt):
            nc.sync.dma_start(dr["out"].ap()[tt * P128:(tt + 1) * P128, :], os_)
        steps.append(out)
        return steps

    def norm_steps(t, tqb, pv):
        # normalize rows 0..63 by row 64, spread over up to 4 chunk slots
        tq0 = tqb * TQB
        pvs = small.tile([65, 1024], F32, tag="pvs", name=f"pvs_{t}_{tqb}")
        ss = small.tile([1, 1024], F32, tag="ss", name=f"ss_{t}_{tqb}",
                        bufs=1)
        r = small.tile([1, 1024], F32, tag="r", name=f"r_{t}_{tqb}", bufs=1)
        rb = small.tile([64, 1024], F32, tag="rb", name=f"rb_{t}_{tqb}")
        h2s = small.tile([64, 512], BF, tag="h2s", name=f"h2s_{t}_{tqb}")

        def s1():
            nc.vector.tensor_copy(pvs, pv[0:65, :])     # frees pv fast

        def s2():
            nc.vector.tensor_copy(ss, pvs[64:65, :])
            nc.vector.reciprocal_approx_fast(out=r, in_=ss)
            nc.gpsimd.partition_broadcast(rb, r, channels=64)

        def s3():
            nc.vector.tensor_mul(
                attn_sb[0:64, t, tq0:tq0 + TQB], pvs[0:64, 0:512],
                rb[:, 0:512])

        def s4():
            nc.vector.tensor_mul(h2s, pvs[0:64, 512:1024], rb[:, 512:1024])
            nc.sync.dma_start(attn_sb[64:128, t, tq0:tq0 + TQB], h2s)
        return [s1, s2, s3, s4]

    # ---- prologue compute: kproj blk0 + qproj (0,0) ----
    for fn in kproj_blk_steps(0, ps_qp, "ps_qp"):
        fn()
    for fn in qproj_chain(0, 0):
        fn()

    # ---- main loop: 16 blocks x 16 chunks; emission per chunk is
    # scores(c) -> exp(c) -> extras -> PV(c-1) ----
    pend_pv = None       # deferred PV step for the previous chunk
    pend_norm = []       # deferred norm steps for the previous block

    for k in range(16):
        tqb, t = k // 4, k % 4
        tq0 = tqb * TQB
        pv = ps_pv.tile([P128, 1024], F32, tag="ps_pv", name=f"pv_{t}_{tqb}")

        # extras: chunk -> list of callables (PE work mostly)
        extras = {c: [] for c in range(NTK)}
        if k == 0:
            # ps_qp emission order: kps0, qps00 (prologue), kps1, qps-j1
            # ps_po emission order: vps0, vps1, kps2, vps2, kps3, vps3
            for i, fn in enumerate(vproj_blk_steps(0)):   # chunks 0-1
                extras[min(i // 5, 1)].append(fn)
            for i, fn in enumerate(kproj_blk_steps(1, ps_qp, "ps_qp")):
                extras[2 + min(i // 5, 1)].append(fn)     # chunks 2-3
            for i, fn in enumerate(vproj_blk_steps(1)):   # chunks 2-3
                extras[2 + min(i // 5, 1)].append(fn)
            for i, fn in enumerate(kproj_blk_steps(2, ps_po, "ps_po")):
                extras[5 + min(i // 5, 1)].append(fn)     # chunks 5-6
            for i, fn in enumerate(vproj_blk_steps(2)):   # chunks 7-8
                extras[7 + min(i // 5, 1)].append(fn)
            for i, fn in enumerate(kproj_blk_steps(3, ps_po, "ps_po")):
                extras[9 + min(i // 5, 1)].append(fn)     # chunks 9-10
            for i, fn in enumerate(vproj_blk_steps(3)):   # chunks 11-12
                extras[11 + min(i // 5, 1)].append(fn)
            qp0 = 4                              # qproj chain chunks 4-12
        else:
            qp0 = 1                              # qproj chain chunks 1-9
        if k + 1 < 16:
            nj, ntqb = (k + 1) % 4, (k + 1) // 4
            for i, fn in enumerate(qproj_chain(nj, ntqb)):
                extras[qp0 + i].append(fn)
        if tqb > 0:
            for i, fn in enumerate(wp_tile(4 * (tqb - 1) + t)):
                extras[5 + i].append(fn)

        for c in range(NTK):
            # scores: head pair via PE row tiling (K=64, partitions 0/64)
            s2 = ps_s2.tile([P128, 1024], F32, tag="ps_s2",
                            name=f"s2_{t}_{tqb}_{c}")
            nc.tensor.matmul(
                s2[:, 0:512],
                k2_sb[0:64, c * P128:(c + 1) * P128],
                qpt_sb[0:64, t, tq0:tq0 + TQB],
                start=True, stop=True)
            nc.tensor.matmul(
                s2[:, 512:1024],
                k2_sb[64:128, c * P128:(c + 1) * P128],
                qpt_sb[64:128, t, tq0:tq0 + TQB],
                start=True, stop=True)
            p = ppool.tile([P128, 1024], BF, tag="p", name=f"p_{t}_{tqb}_{c}")
            nc.scalar.activation(p, s2, EXP, scale=SCALE)
            # extras first (block 0's vproj feeds PV), then the previous
            # chunk's PV, then the norm step that reads the finished pv
            for fn in extras[c]:
                fn()
            if pend_pv is not None:
                pend_pv()
            if pend_norm:
                pend_norm.pop(0)()

            def pv_step(p=p, c=c, pv=pv):
                nc.tensor.matmul(
                    pv[0:65, 0:512], v65_sb[:, c, :], p[:, 0:512],
                    start=(c == 0), stop=(c == NTK - 1))
                nc.tensor.matmul(
                    pv[0:65, 512:1024], v65_sb[:, c, :], p[:, 512:1024],
                    start=(c == 0), stop=(c == NTK - 1))
            pend_pv = pv_step
        pend_norm = norm_steps(t, tqb, pv)

    # ---- tail: last PV + norm, then wp tiles 12-15 (2-way via idle s2
    # banks) and the final output stores ----
    pend_pv()
    for fn in pend_norm:
        fn()
    for tt in range(12, 16):
        po = ps_s2.tile([P128, 1024], F32, tag="ps_s2", name=f"pot_{tt}")
        for rr in range(HD // P128):
            lhsT = attn_sb[:, rr, tt * P128:(tt + 1) * P128]
            nc.tensor.matmul(po[:, 0:512], lhsT, wp_sb[:, rr, 0:512],
                             start=(rr == 0), stop=(rr == 3))
            nc.tensor.matmul(po[:, 512:1024], lhsT, wp_sb[:, rr, 512:1024],
                             start=(rr == 0), stop=(rr == 3))
        os_ = outp.tile([P128, 1024], F32, tag="os", name=f"ost_{tt}")
        nc.vector.tensor_copy(os_, po)
        nc.sync.dma_start(dr["out"].ap()[tt * P128:(tt + 1) * P128, :], os_)

    if DEBUG:
        nc.sync.dma_start(dr["dbg_k2"].ap(), k2_sb)
        nc.sync.dma_start(dr["dbg_v65"].ap(),
                          v65_sb.rearrange("p c d -> p (c d)"))
        nc.sync.dma_start(dr["dbg_qpt"].ap(),
                          qpt_sb.rearrange("p j t -> p (j t)"))
        nc.sync.dma_start(dr["dbg_attn"].ap(),
                          attn_sb.rearrange("p j t -> p (j t)"))


def build_nc():
    nc = bacc.Bacc("TRN2", target_bir_lowering=False, debug=False)
    dr = {
        "qT": nc.dram_tensor("qT", [C, T], BF, kind="ExternalInput"),
        "kT": nc.dram_tensor("kT", [C, T], BF, kind="ExternalInput"),
        "vT": nc.dram_tensor("vT", [C, T], BF, kind="ExternalInput"),
        "wq": nc.dram_tensor("wq", [C, HD], BF, kind="ExternalInput"),
        "wk2": nc.dram_tensor("wk2", [C, P128], BF, kind="ExternalInput"),
        "wv": nc.dram_tensor("wv", [C, D], BF, kind="ExternalInput"),
        "wp": nc.dram_tensor("wp", [HD, C], BF, kind="ExternalInput"),
        "out": nc.dram_tensor("out", [T, C], F32, kind="ExternalOutput"),
    }
    if DEBUG:
        dr["dbg_k2"] = nc.dram_tensor("dbg_k2", [P128, T], BF,
                                      kind="ExternalOutput")
        dr["dbg_v65"] = nc.dram_tensor("dbg_v65", [P128, NTK * (D + 1)], BF,
                                       kind="ExternalOutput")
        dr["dbg_qpt"] = nc.dram_tensor("dbg_qpt", [P128, NTP * T], BF,
                                       kind="ExternalOutput")
        dr["dbg_attn"] = nc.dram_tensor("dbg_attn", [P128, NTP * T], BF,
                                        kind="ExternalOutput")
    with tile.TileContext(nc) as tc, ExitStack() as ctx:
        emit_kernel(ctx, tc, dr)
    nc.compile()
    return nc


_NC_CACHE = None


def _get_nc():
    global _NC_CACHE
    if _NC_CACHE is None:
        _NC_CACHE = build_nc()
    return _NC_CACHE


def make_in_maps(q, k, v, Wq, Wk, Wv, Wp):
    """Per-core input dicts (host-side sharding + transpose + bf16 cast)."""
    bf = lambda x: np.ascontiguousarray(x).astype(NPBF)
    wk2 = np.concatenate([Wk, Wk], axis=1)
    per_b = []
    for b in range(B):
        per_b.append((bf(q[b].T), bf(k[b].T), bf(v[b].T)))
    in_maps = []
    for core in range(NCORES):
        b, g = core // 2, core % 2
        qT, kT, vT = per_b[b]
        in_maps.append({
            "qT": qT, "kT": kT, "vT": vT,
            "wq": bf(Wq[:, g * HD:(g + 1) * HD]),
            "wk2": bf(wk2),
            "wv": bf(Wv),
            "wp": bf(Wp[g * HD:(g + 1) * HD, :]),
        })
    return in_maps


def kernel(q, k, v, Wq, Wk, Wv, Wp, bp):
    from concourse.bass_utils import run_bass_kernel_spmd

    q, k, v, Wq, Wk, Wv, Wp, bp = (np.asarray(x, np.float32)
                                   for x in (q, k, v, Wq, Wk, Wv, Wp, bp))
    nc = _get_nc()
    in_maps = make_in_maps(q, k, v, Wq, Wk, Wv, Wp)
    res = run_bass_kernel_spmd(nc, in_maps, list(range(NCORES))).results
    out = np.empty((B, T, C), np.float32)
    for b in range(B):
        out[b] = res[2 * b]["out"] + res[2 * b + 1]["out"] + bp
    return out


# revision 21
# speedup vs baseline: 1.0094x; 1.0094x over previous
"""MultiQueryAttention Trainium2 kernel (8 NeuronCores, SPMD).

Reference computation (per batch b):
    q_proj = q @ Wq            [T, C] -> [T, H, D]   (H=16 heads, D=64)
    k_proj = k @ Wk            [T, D]   (single shared KV head)
    v_proj = v @ Wv            [T, D]
    S_h    = q_h @ k_proj.T / sqrt(D)      [T, T] per head
    P      = softmax(S)        (no mask)
    out    = (P @ v_proj  for each head) -> [T, C]; out @ Wp + bp

Sharding: 8 cores = batch (4) x head-halves (2). Each core handles one
batch and 8 query heads; the shared K/V projections are replicated.
Wq is split column-wise, Wp row-wise; each pair of cores produces a
partial [T, C] output that the host sums (+ bp).

Device layout notes:
  - All matmul operands are bf16 (PE streams bf16 at 1 cyc/row vs 2 for
    fp32); PSUM accumulation is fp32.
  - Host pre-transposes q/k/v to [C, T] so every projection contraction
    (over C) has C on the partition axis.
  - Scores are computed transposed: S^T[tk, tq] so that P^T can feed the
    P@V matmul directly as the stationary operand.  The two heads of a
    head-pair run concurrently in the PE array via row tiling (K=64 each,
    base partitions 0 and 64).
  - Row-sums of P come for free from a ones-column appended to v_proj
    (stationary [v | 1] -> output row 64 is the softmax denominator).
  - softmax(x) is computed without max-subtraction: scores are ~N(0, 0.4)
    here so exp is safe in fp32, and the reference's max-subtraction is
    mathematically a no-op.

Schedule notes (v2):
  - The ACT engine is the bottleneck: 256 exp instructions of [128,1024]
    at ~(N+352)/1.2ns each = ~280us busy.  Per chunk we emit
    scores(c) -> exp(c) -> [extras] -> PV(c-1): the PV of the PREVIOUS
    chunk runs after the next scores pair, so the in-order PE queue never
    makes ACT wait for a score pair stuck behind PV matmuls.
  - Input DMAs are ordered by first use at 512-tk granularity, and
    kproj/vproj are emitted per 512-tk block interleaved into block 0, so
    the first exp fires as soon as ~2.5MB (not 6.5MB) has landed.
  - vproj is computed transposed (wv stationary, vt moving, N=512) which
    is 4x fewer PE slots than the naive [tk,d] form; a small SBUF->SBUF
    DMA transpose (plus a column-interleaving DVE copy) restores the
    [tk, d] layout the PV stationary needs.
  - The softmax reciprocal is partition-broadcast on the (otherwise idle)
    GPSIMD engine instead of a DRAM round-trip DMA bounce.
"""

import numpy as np
import ml_dtypes
from contextlib import ExitStack

import concourse.bacc as bacc
import concourse.bass as bass
import concourse.mybir as mybir
import concourse.tile as tile

B, T, C = 4, 2048, 1024
H, D = 16, 64
HPC = 8              # heads per core
HD = HPC * D         # 512 per-core attention output dims
NCORES = 8
P128 = 128
NCC = C // P128      # 8 contraction chunks over C
NTK = T // P128      # 16 key chunks
NTQB = 4             # query blocks of 512
TQB = 512
NTP = 4              # head-pairs per core
SCALE = 1.0 / 8.0    # 1/sqrt(64)

BF = mybir.dt.bfloat16
F32 = mybir.dt.float32
NPBF = ml_dtypes.bfloat16
DEBUG = False      # adds intermediate dumps (k2/v65/qpt/attn) as outputs


def emit_kernel(ctx: ExitStack, tc: tile.TileContext, dr):
    nc = tc.nc
    EXP = mybir.ActivationFunctionType.Exp

    const = ctx.enter_context(tc.tile_pool(name="const", bufs=1))
    persist = ctx.enter_context(tc.tile_pool(name="persist", bufs=1))
    stream = ctx.enter_context(tc.tile_pool(name="stream", bufs=2))
    ppool = ctx.enter_context(tc.tile_pool(name="ppool", bufs=7))
    small = ctx.enter_context(tc.tile_pool(name="small", bufs=2))
    outp = ctx.enter_context(tc.tile_pool(name="outp", bufs=2))
    # PSUM budget (8 banks): s2 rotation 2x2 + pv 2 + qp 1 + po 1
    ps_s2 = ctx.enter_context(tc.tile_pool(name="ps_s2", bufs=2, space="PSUM"))
    ps_pv = ctx.enter_context(tc.tile_pool(name="ps_pv", bufs=1, space="PSUM"))
    ps_qp = ctx.enter_context(tc.tile_pool(name="ps_qp", bufs=1, space="PSUM"))
    ps_po = ctx.enter_context(tc.tile_pool(name="ps_po", bufs=1, space="PSUM"))

    kT_r = dr["kT"].ap().rearrange("(cc p) t -> p cc t", p=P128)
    qT_r = dr["qT"].ap().rearrange("(cc p) t -> p cc t", p=P128)
    vT_r = dr["vT"].ap().rearrange("(cc p) t -> p cc t", p=P128)
    wq_r = dr["wq"].ap().rearrange("(cc p) d -> p cc d", p=P128)

    # ---- input DMAs, few and large, ordered by first use ----
    # first-exp critical set: wk2, wq, kT[:,0:1024], qT[:,0:512] (~4.25MB)
    wk2_sb = const.tile([P128, NCC, P128], BF)       # Wk duplicated -> [*, 128]
    nc.sync.dma_start(wk2_sb, dr["wk2"].ap().rearrange("(cc p) d -> p cc d", p=P128))
    wq_sb = const.tile([P128, NCC, HD], BF)          # [c-in-chunk, cc, dcol]
    nc.sync.dma_start(wq_sb, wq_r)
    kts = []
    for cc in range(NCC):
        kt = stream.tile([P128, T], BF, tag="kv_chunk", name=f"kt{cc}", bufs=8)
        nc.sync.dma_start(kt[:, 0:1024], kT_r[:, cc, 0:1024])
        kts.append(kt)
    qt_sb = persist.tile([P128, NCC, T], BF)
    for cc in range(NCC):
        nc.sync.dma_start(qt_sb[:, cc, 0:TQB], qT_r[:, cc, 0:TQB])
    # block-0 support, in data-need order
    wv_sb = const.tile([P128, NCC, D], BF)
    nc.sync.dma_start(wv_sb, dr["wv"].ap().rearrange("(cc p) d -> p cc d", p=P128))
    vt_sb = stream.tile([P128, NCC, T], BF, tag="vt_all", bufs=1)
    nc.sync.dma_start(vt_sb[:, :, 0:512], vT_r[:, :, 0:512])
    nc.sync.dma_start(vt_sb[:, :, 512:1024], vT_r[:, :, 512:1024])
    for cc in range(NCC):
        nc.sync.dma_start(kts[cc][:, 1024:2048], kT_r[:, cc, 1024:2048])
    nc.sync.dma_start(vt_sb[:, :, 1024:1536], vT_r[:, :, 1024:1536])
    nc.sync.dma_start(vt_sb[:, :, 1536:2048], vT_r[:, :, 1536:2048])
    for cc in range(NCC):
        nc.sync.dma_start(qt_sb[:, cc, TQB:T], qT_r[:, cc, TQB:T])
    wp_sb = const.tile([P128, HD // P128, C], BF)    # [hd-in-chunk, r, c-out]
    nc.sync.dma_start(wp_sb, dr["wp"].ap().rearrange("(r p) c -> p r c", p=P128))

    # ---- K projection: k2[0:64]=k_projT, k2[64:128]=k_projT (dup), one
    # 512-tk block at a time so scores gate at blk granularity ----
    k2_sb = persist.tile([P128, T], BF)

    def kproj_blk_steps(b, pool, tag):
        kps = pool.tile([P128, 512], F32, tag=tag, name=f"kps{b}")

        def mm(cc):
            nc.tensor.matmul(kps, wk2_sb[:, cc, :],
                             kts[cc][:, b * 512:(b + 1) * 512],
                             start=(cc == 0), stop=(cc == NCC - 1))

        def fin():
            nc.vector.tensor_copy(k2_sb[:, b * 512:(b + 1) * 512], kps)
        return [lambda cc=cc: mm(cc) for cc in range(NCC)] + [fin]

    # ---- V projection, transposed: vps[d 64, tk 512] = wv.T @ vt_blk,
    # then a column-interleaving copy + DMA transpose into v65 [tk, d] ----
    v65_sb = persist.tile([P128, NTK, D + 1], BF)
    nc.vector.memset(v65_sb[:, :, D:D + 1], 1.0)
    v2t_sb = persist.tile([D, T], BF)

    def vproj_blk_steps(b):
        vps = ps_po.tile([P128, 512], F32, tag="ps_po", name=f"vps{b}")

        def mm(cc):
            nc.tensor.matmul(vps[0:D, :], wv_sb[:, cc, :],
                             vt_sb[:, cc, b * 512:(b + 1) * 512],
                             start=(cc == 0), stop=(cc == NCC - 1))

        def fin():
            # xbar transpose: out[p, c, d] = v2t[d, c*128+p], but it needs a
            # contiguous dest tile; a small DVE copy lands it in v65's
            # 65-element-pitch layout.
            nc.vector.tensor_copy(v2t_sb[:, b * 512:(b + 1) * 512],
                                  vps[0:D, :])
            vtr = small.tile([P128, 4, D], BF, tag="vtr", name=f"vtr{b}")
            # ACT queue: keeps the transpose off the (in-order) sync queue,
            # which is deep in backpressured bulk input DMAs during block 0
            nc.scalar.dma_start_transpose(
                vtr, v2t_sb[:, b * 512:(b + 1) * 512])
            nc.vector.tensor_copy(v65_sb[:, b * 4:(b + 1) * 4, 0:D], vtr)
        return [lambda cc=cc: mm(cc) for cc in range(NCC)] + [fin]

    # ---- Q projection: one (dcol, tq-block) chain ----
    qpt_sb = persist.tile([P128, NTP, T], BF)

    def qproj_chain(j, tqb):
        qps = ps_qp.tile([P128, 512], F32, tag="ps_qp", name=f"qps_{j}_{tqb}")

        def mm(cc):
            nc.tensor.matmul(
                qps, wq_sb[:, cc, j * P128:(j + 1) * P128],
                qt_sb[:, cc, tqb * 512:(tqb + 1) * 512],
                start=(cc == 0), stop=(cc == NCC - 1))

        def fin():
            nc.vector.tensor_copy(
                qpt_sb[:, j, tqb * 512:(tqb + 1) * 512], qps)
        return [lambda cc=cc: mm(cc) for cc in range(NCC)] + [fin]

    attn_sb = persist.tile([P128, NTP, T], BF)   # attn_outT (normalized), bf16

    def wp_tile(tt):
        # two sequential half-chains through one PSUM bank
        po = ps_po.tile([P128, 512], F32, tag="ps_po", name=f"po_{tt}")
        os_ = outp.tile([P128, 1024], F32, tag="os", name=f"os_{tt}")
        steps = []
        for half in range(2):
            for rr in range(HD // P128):
                def mm(rr=rr, half=half):
                    nc.tensor.matmul(
                        po, attn_sb[:, rr, tt * P128:(tt + 1) * P128],
                        wp_sb[:, rr, half * 512:half * 512 + 512],
                        start=(rr == 0), stop=(rr == 3))
                steps.append(mm)

            def cp(half=half):
                nc.vector.tensor_copy(os_[:, half * 512:half * 512 + 512], po)
            steps.append(cp)

        def out(tt=tt):
            nc.sync.dma_start(dr["out"].ap()[tt * P128:(tt + 1) * P128, :], os_)
        steps.append(out)
        return steps

    def norm_steps(t, tqb, pv):
        # normalize rows 0..63 by row 64, spread over up to 4 chunk slots
        tq0 = tqb * TQB
        pvs = small.tile([65, 1024], F32, tag="pvs", name=f"pvs_{t}_{tqb}")
        ss = small.tile([1, 1024], F32, tag="ss", name=f"ss_{t}_{tqb}",
                        bufs=1)
        r = small.tile([1, 1024], F32, tag="r", name=f"r_{t}_{tqb}", bufs=1)
        rb = small.tile([64, 1024], F32, tag="rb", name=f"rb_{t}_{tqb}")
        h2s = small.tile([64, 512], BF, tag="h2s", name=f"h2s_{t}_{tqb}")

        def s1():
            nc.vector.tensor_copy(pvs, pv[0:65, :])     # frees pv fast

        def s2():
            nc.vector.tensor_copy(ss, pvs[64:65, :])
            nc.vector.reciprocal_approx_fast(out=r, in_=ss)
            nc.gpsimd.partition_broadcast(rb, r, channels=64)

        def s3():
            nc.vector.tensor_mul(
                attn_sb[0:64, t, tq0:tq0 + TQB], pvs[0:64, 0:512],
                rb[:, 0:512])

        def s4():
            nc.vector.tensor_mul(h2s, pvs[0:64, 512:1024], rb[:, 512:1024])
            nc.sync.dma_start(attn_sb[64:128, t, tq0:tq0 + TQB], h2s)
        return [s1, s2, s3, s4]

    # ---- prologue compute: kproj blk0 + qproj (0,0) ----
    for fn in kproj_blk_steps(0, ps_qp, "ps_qp"):
        fn()
    for fn in qproj_chain(0, 0):
        fn()

    # ---- main loop: 16 blocks x 16 chunks; emission per chunk is
    # scores(c) -> exp(c) -> extras -> PV(c-1) ----
    pend_pv = None       # deferred PV step for the previous chunk
    pend_norm = []       # deferred norm steps for the previous block

    for k in range(16):
        tqb, t = k // 4, k % 4
        tq0 = tqb * TQB
        pv = ps_pv.tile([P128, 1024], F32, tag="ps_pv", name=f"pv_{t}_{tqb}")

        # extras: chunk -> list of callables (PE work mostly)
        extras = {c: [] for c in range(NTK)}
        if k == 0:
            # deadline-driven: kps-blk-b fin before chunk 4b (scores gate),
            # vps-blk-b fin before chunk 4b+1 (the lagged PV gate).
            # ps_qp emission order: kps0, qps00 (prologue), kps1, qps-j1
            # ps_po emission order: vps0, vps1, kps2, vps2, kps3, vps3
            for i, fn in enumerate(vproj_blk_steps(0)):   # chunks 0-1
                extras[min(i // 5, 1)].append(fn)
            for i, fn in enumerate(kproj_blk_steps(1, ps_qp, "ps_qp")):
                extras[2 + min(i // 5, 1)].append(fn)     # chunks 2-3
            for i, fn in enumerate(vproj_blk_steps(1)):   # chunks 3-4
                extras[3 + min(i // 5, 1)].append(fn)
            for i, fn in enumerate(kproj_blk_steps(2, ps_po, "ps_po")):
                extras[6 + min(i // 5, 1)].append(fn)     # chunks 6-7
            for i, fn in enumerate(vproj_blk_steps(2)):   # chunks 7-8
                extras[7 + min(i // 5, 1)].append(fn)
            for i, fn in enumerate(kproj_blk_steps(3, ps_po, "ps_po")):
                extras[9 + min(i // 5, 1)].append(fn)     # chunks 9-10
            for i, fn in enumerate(vproj_blk_steps(3)):   # chunks 11-12
                extras[11 + min(i // 5, 1)].append(fn)
            qp0 = 5                              # qproj chain chunks 5-13
        else:
            qp0 = 1                              # qproj chain chunks 1-9
        if k + 1 < 16:
            nj, ntqb = (k + 1) % 4, (k + 1) // 4
            for i, fn in enumerate(qproj_chain(nj, ntqb)):
                extras[qp0 + i].append(fn)
        if tqb > 0:
            for i, fn in enumerate(wp_tile(4 * (tqb - 1) + t)):
                extras[5 + i].append(fn)

        for c in range(NTK):
            # scores: head pair via PE row tiling (K=64, partitions 0/64)
            s2 = ps_s2.tile([P128, 1024], F32, tag="ps_s2",
                            name=f"s2_{t}_{tqb}_{c}")
            nc.tensor.matmul(
                s2[:, 0:512],
                k2_sb[0:64, c * P128:(c + 1) * P128],
                qpt_sb[0:64, t, tq0:tq0 + TQB],
                start=True, stop=True)
            nc.tensor.matmul(
                s2[:, 512:1024],
                k2_sb[64:128, c * P128:(c + 1) * P128],
                qpt_sb[64:128, t, tq0:tq0 + TQB],
                start=True, stop=True)
            p = ppool.tile([P128, 1024], BF, tag="p", name=f"p_{t}_{tqb}_{c}")
            nc.scalar.activation(p, s2, EXP, scale=SCALE)
            # extras first (block 0's vproj feeds PV), then the previous
            # chunk's PV, then the norm step that reads the finished pv
            for fn in extras[c]:
                fn()
            if pend_pv is not None:
                pend_pv()
            if pend_norm:
                pend_norm.pop(0)()

            def pv_step(p=p, c=c, pv=pv):
                nc.tensor.matmul(
                    pv[0:65, 0:512], v65_sb[:, c, :], p[:, 0:512],
                    start=(c == 0), stop=(c == NTK - 1))
                nc.tensor.matmul(
                    pv[0:65, 512:1024], v65_sb[:, c, :], p[:, 512:1024],
                    start=(c == 0), stop=(c == NTK - 1))
            pend_pv = pv_step
        pend_norm = norm_steps(t, tqb, pv)

    # ---- tail: last PV + norm, then wp tiles 12-15 (2-way via idle s2
    # banks) and the final output stores ----
    pend_pv()
    for fn in pend_norm:
        fn()
    for tt in range(12, 16):
        po = ps_s2.tile([P128, 1024], F32, tag="ps_s2", name=f"pot_{tt}")
        for rr in range(HD // P128):
            lhsT = attn_sb[:, rr, tt * P128:(tt + 1) * P128]
            nc.tensor.matmul(po[:, 0:512], lhsT, wp_sb[:, rr, 0:512],
                             start=(rr == 0), stop=(rr == 3))
            nc.tensor.matmul(po[:, 512:1024], lhsT, wp_sb[:, rr, 512:1024],
                             start=(rr == 0), stop=(rr == 3))
        os_ = outp.tile([P128, 1024], F32, tag="os", name=f"ost_{tt}")
        nc.vector.tensor_copy(os_, po)
        nc.sync.dma_start(dr["out"].ap()[tt * P128:(tt + 1) * P128, :], os_)

    if DEBUG:
        nc.sync.dma_start(dr["dbg_k2"].ap(), k2_sb)
        nc.sync.dma_start(dr["dbg_v65"].ap(),
                          v65_sb.rearrange("p c d -> p (c d)"))
        nc.sync.dma_start(dr["dbg_qpt"].ap(),
                          qpt_sb.rearrange("p j t -> p (j t)"))
        nc.sync.dma_start(dr["dbg_attn"].ap(),
                          attn_sb.rearrange("p j t -> p (j t)"))


def build_nc():
    nc = bacc.Bacc("TRN2", target_bir_lowering=False, debug=False)
    dr = {
        "qT": nc.dram_tensor("qT", [C, T], BF, kind="ExternalInput"),
        "kT": nc.dram_tensor("kT", [C, T], BF, kind="ExternalInput"),
        "vT": nc.dram_tensor("vT", [C, T], BF, kind="ExternalInput"),
        "wq": nc.dram_tensor("wq", [C, HD], BF, kind="ExternalInput"),
        "wk2": nc.dram_tensor("wk2", [C, P128], BF, kind="ExternalInput"),
        "wv": nc.dram_tensor("wv", [C, D], BF, kind="ExternalInput"),
        "wp": nc.dram_tensor("wp", [HD, C], BF, kind="ExternalInput"),
        "out": nc.dram_tensor("out", [T, C], F32, kind="ExternalOutput"),
    }
    if DEBUG:
        dr["dbg_k2"] = nc.dram_tensor("dbg_k2", [P128, T], BF,
                                      kind="ExternalOutput")
        dr["dbg_v65"] = nc.dram_tensor("dbg_v65", [P128, NTK * (D + 1)], BF,
                                       kind="ExternalOutput")
        dr["dbg_qpt"] = nc.dram_tensor("dbg_qpt", [P128, NTP * T], BF,
                                       kind="ExternalOutput")
        dr["dbg_attn"] = nc.dram_tensor("dbg_attn", [P128, NTP * T], BF,
                                        kind="ExternalOutput")
    with tile.TileContext(nc) as tc, ExitStack() as ctx:
        emit_kernel(ctx, tc, dr)
    nc.compile()
    return nc


_NC_CACHE = None


def _get_nc():
    global _NC_CACHE
    if _NC_CACHE is None:
        _NC_CACHE = build_nc()
    return _NC_CACHE


def make_in_maps(q, k, v, Wq, Wk, Wv, Wp):
    """Per-core input dicts (host-side sharding + transpose + bf16 cast)."""
    bf = lambda x: np.ascontiguousarray(x).astype(NPBF)
    wk2 = np.concatenate([Wk, Wk], axis=1)
    per_b = []
    for b in range(B):
        per_b.append((bf(q[b].T), bf(k[b].T), bf(v[b].T)))
    in_maps = []
    for core in range(NCORES):
        b, g = core // 2, core % 2
        qT, kT, vT = per_b[b]
        in_maps.append({
            "qT": qT, "kT": kT, "vT": vT,
            "wq": bf(Wq[:, g * HD:(g + 1) * HD]),
            "wk2": bf(wk2),
            "wv": bf(Wv),
            "wp": bf(Wp[g * HD:(g + 1) * HD, :]),
        })
    return in_maps


def kernel(q, k, v, Wq, Wk, Wv, Wp, bp):
    from concourse.bass_utils import run_bass_kernel_spmd

    q, k, v, Wq, Wk, Wv, Wp, bp = (np.asarray(x, np.float32)
                                   for x in (q, k, v, Wq, Wk, Wv, Wp, bp))
    nc = _get_nc()
    in_maps = make_in_maps(q, k, v, Wq, Wk, Wv, Wp)
    res = run_bass_kernel_spmd(nc, in_maps, list(range(NCORES))).results
    out = np.empty((B, T, C), np.float32)
    for b in range(B):
        out[b] = res[2 * b]["out"] + res[2 * b + 1]["out"] + bp
    return out
